# revision 1
# baseline (speedup 1.0000x reference)
"""Trainium2 Bass kernel for nn_Attention_FRN (sparse windowed attention argmax).

Math: reference computes
    q  = (HSI flat -> (B,L,C)) @ Wq          (Wq = W_qkv1[:, :C])
    k  = (MSI flat -> (B,9L,C)) @ Wk         (Wk = W_qkv2[:, C:2C])
    attn[b,l,n] = <q[b,l], k[b,9l+n]> * scale
    out = argmax_n softmax(attn)  -> (n//3-1, n%3-1) offsets, reshaped (B,H,H,2)

Softmax and the positive scale are argmax-invariant, so both are skipped.
x and y inputs are unused (only x's shape matters).  The big k projection is
eliminated algebraically:  <q[l], mp[s] @ Wk> = <(q @ Wk^T)[l], mp[s]>, so we
only project the small hp tensor:  qm = (hp @ Wq) @ Wk^T, then
attn[l, n] = <qm[l], mp[9l+n]> -- a banded dot handled on the tensor engine.

Sharding: 8 cores = B(4) x L-half(2).  Per core: hp shard (256, 2048),
mp shard (256, 18432), replicated 256x256 weight slices + small constants.
"""

import numpy as np

B, C, H = 4, 256, 64
L = H * H                  # 4096 pixels per batch
L_SH = L // 2              # 2048 per core
S_SH = 9 * L_SH            # 18432 mp columns per core
NCHUNK = L_SH // 128       # 16 chunks of 128 pixels
WIN = 288                  # 9 * 32: s-window per 32-pixel group
MP_TW = 4608               # mp tile width = 4 chunks * 1152
NEG = -1.0e30

_CACHE = {}


def _build_nc():
    import os

    import concourse.bacc as bacc
    import concourse.tile as tile
    from concourse import mybir

    f32 = mybir.dt.float32
    u32 = mybir.dt.uint32
    # fp32r shares numpy float32 layout; declaring mp as f32r feeds the
    # single-pass reduced-precision matmul path without any cast DMA.
    use_f32r = os.environ.get("KF32R", "attn") in ("attn", "all")
    mp_dt = mybir.dt.float32r if use_f32r else f32

    nc = bacc.Bacc(
        "TRN2",
        target_bir_lowering=False,
        debug=False,
        enable_asserts=False,
        num_devices=8,
    )
    hp_d = nc.dram_tensor("hp", [C, L_SH], mp_dt, kind="ExternalInput").ap()
    mp_d = nc.dram_tensor("mp", [C, S_SH], mp_dt, kind="ExternalInput").ap()
    wqt_d = nc.dram_tensor("wqt", [C, C], mp_dt, kind="ExternalInput").ap()
    wkt_d = nc.dram_tensor("wkt", [C, C], mp_dt, kind="ExternalInput").ap()
    bm_d = nc.dram_tensor("bandmask", [128, WIN], f32, kind="ExternalInput").ap()
    bm4_d = nc.dram_tensor("bandmask4", [128, 2 * WIN], mp_dt, kind="ExternalInput").ap()
    id2_d = nc.dram_tensor("ident_r", [128, 128], mp_dt, kind="ExternalInput").ap()
    b9_d = nc.dram_tensor("base9", [128, NCHUNK], f32, kind="ExternalInput").ap()
    out_d = nc.dram_tensor("out", [128, 2 * NCHUNK], f32, kind="ExternalOutput").ap()
    # Top-2 attn values per pixel: lets the host exactly re-resolve near-ties.
    m2_d = nc.dram_tensor("m2", [128, 2 * NCHUNK], f32, kind="ExternalOutput").ap()

    with tile.TileContext(nc) as tc:
        _emit(tc, out_d, m2_d, hp_d, mp_d, wqt_d, wkt_d, bm_d, bm4_d, id2_d, b9_d)
    nc.compile()
    return nc


def _emit(tc, out_d, m2_d, hp_d, mp_d, wqt_d, wkt_d, bm_d, bm4_d, id2_d, b9_d):
    import os
    from contextlib import ExitStack

    from concourse import mybir

    # Debug switches for cost-model bisection (default: full kernel).
    SKIP = set(os.environ.get("KSKIP", "").split(","))
    # fp32r (single-pass reduced-precision fp32 matmul, 4x faster):
    #   "attn" = banded attention matmuls only, "all" = also q projections.
    F32R = os.environ.get("KF32R", "attn")
    # Band-mask application: "mm" = extra identity matmul into PSUM + ACT
    # copy to SBUF; "stt" = fused DVE scalar_tensor_tensor add.
    MASK = os.environ.get("KMASK", "stt")
    # mp stream tile width (multiple of 1152) and buffer count.
    MPTW = int(os.environ.get("KMPTW", "1152"))
    MPBUFS = int(os.environ.get("KMPBUFS", "10"))

    nc = tc.nc
    f32 = mybir.dt.float32
    f32r = mybir.dt.float32r
    u32 = mybir.dt.uint32
    AL = mybir.AluOpType
    dma = nc.sync.dma_start          # SP HWDGE ring: the mp stream
    dma_aux = nc.scalar.dma_start    # ACT HWDGE ring: consts/hp/out

    use_f32r = F32R in ("attn", "all")
    mp_dt = f32r if use_f32r else f32

    with ExitStack() as ctx:
        consts = ctx.enter_context(tc.tile_pool(name="consts", bufs=1))
        mpp = ctx.enter_context(tc.tile_pool(name="mpp", bufs=MPBUFS))
        zsb = ctx.enter_context(tc.tile_pool(name="zsb", bufs=int(os.environ.get("KZSB", "3"))))
        setup_ctx = ExitStack()
        psA = setup_ctx.enter_context(tc.tile_pool(name="psA", bufs=2, space="PSUM"))

        wqT_sb = consts.tile([128, 2, C], mp_dt)  # (k % 128, k // 128, c1)
        wkT_sb = consts.tile([128, 2, C], mp_dt)  # (k % 128, k // 128, c2)
        m_sb = consts.tile([128, 2, C], mp_dt)    # (c1 % 128, c1 // 128, c2)
        hp_sb = consts.tile([128, 2, L_SH], mp_dt)  # (c1 % 128, c1 // 128, l)
        qmT_sb = consts.tile([128, 2, L_SH], mp_dt)  # (c2 % 128, c2 // 128, l)
        bm_sb = consts.tile([128, WIN], f32)
        bm4_sb = consts.tile([128, 2 * WIN], mp_dt)
        id2_sb = consts.tile([128, 128], mp_dt)
        b9_sb = consts.tile([128, NCHUNK], f32)
        idx8 = consts.tile([128, 8 * NCHUNK], u32)
        m8buf = consts.tile([128, 8 * NCHUNK], f32)
        stage = consts.tile([128, NCHUNK, 2], f32)

        # Order matters: weight/identity DMAs unblock the transpose+M chain,
        # then hp unblocks qmT; bandmask/base9 aren't needed until chunk 0.
        dma_aux(out=wqT_sb[:], in_=wqt_d.rearrange("(c p) k -> p c k", p=128))
        dma_aux(out=wkT_sb[:], in_=wkt_d.rearrange("(c p) k -> p c k", p=128))
        dma_aux(out=id2_sb[:], in_=id2_d)
        dma_aux(out=bm4_sb[:], in_=bm4_d)
        dma_aux(out=bm_sb[:], in_=bm_d)
        dma_aux(out=b9_sb[:], in_=b9_d)
        hp_r = hp_d.rearrange("(c p) l -> p c l", p=128)
        for ls in range(4):
            dma_aux(out=hp_sb[:, :, ls * 512:(ls + 1) * 512],
                    in_=hp_r[:, :, ls * 512:(ls + 1) * 512])

        # M[c1, c2] = sum_k Wq[c1, k] * Wk[c2, k]  (so attn = <hp @ M, mp>)
        for c1 in range(2):
            mps = psA.tile([128, 256], f32, tag="qp")
            for kc in range(2):
                nc.tensor.matmul(
                    mps[:],
                    wqT_sb[:, kc, c1 * 128:(c1 + 1) * 128],
                    wkT_sb[:, kc, :],
                    start=(kc == 0),
                    stop=(kc == 1),
                )
            nc.scalar.copy(out=m_sb[:, c1, :], in_=mps[:])

        # qmT[c2, l] = sum_c1 M[c1, c2] * hpT[c1, l]
        for cc in range(2):
            for ls in range(4):
                qp = psA.tile([128, 512], f32, tag="qp")
                for c1 in range(2):
                    nc.tensor.matmul(
                        qp[:],
                        m_sb[:, c1, cc * 128:(cc + 1) * 128],
                        hp_sb[:, c1, ls * 512:(ls + 1) * 512],
                        start=(c1 == 0),
                        stop=(c1 == 1),
                    )
                # This copy also rounds to fp32r when use_f32r.
                nc.scalar.copy(out=qmT_sb[:, cc, ls * 512:(ls + 1) * 512], in_=qp[:])

        # Close the setup PSUM pool so the main loop can use all 8 banks.
        setup_ctx.close()
        psZ = ctx.enter_context(tc.tile_pool(name="psZ", bufs=2, space="PSUM"))

        # Main loop: stream mp, banded dots on PE, mask+argmax on DVE.
        # fp32r matmul output must start at partition 0, so each chunk uses a
        # full 128-wide stationary (qmT slice) and splits the 1152-wide
        # s-window into 4 matmuls of 288 cols, one per PSUM bank slice of
        # zp4 (the 512-col pitch keeps each slice inside one 2KB bank).
        # Partition p's 9-wide band lands in slice h = p//32 at local offset
        # 9*(p%32), so assembly is 4 partition-aligned fused mask-adds.
        mp_r = mp_d.rearrange("(c p) s -> p c s", p=128)
        assert S_SH % MPTW == 0 and MPTW % 1152 == 0
        n_tiles = S_SH // MPTW
        for it in range(n_tiles):
            last = it == n_tiles - 1 and MPTW == 1152
            if last:
                # Final tile split in half: the closing chunk's first matmuls
                # start as soon as the first 576 columns land.
                mp_a = mpp.tile([128, 2, MPTW], mp_dt, tag="mp_t")
                dma(out=mp_a[:, :, 0:576],
                    in_=mp_r[:, :, it * MPTW:it * MPTW + 576])
                dma(out=mp_a[:, :, 576:1152],
                    in_=mp_r[:, :, it * MPTW + 576:(it + 1) * MPTW])
                mp_t = mp_a
            else:
                mp_t = mpp.tile([128, 2, MPTW], mp_dt, tag="mp_t")
                dma(out=mp_t[:], in_=mp_r[:, :, it * MPTW:(it + 1) * MPTW])
            for j in range(MPTW // 1152):
                i = it * (MPTW // 1152) + j
                # Slices h < n_act get the band mask added in PSUM by an
                # identity matmul and an ACT copy to SBUF; the rest use a
                # fused DVE mask-add.  Balances PE/ACT/DVE load.
                n_act = min(2, int(os.environ.get("KACTH", "2")))
                if os.environ.get("KZSPLIT", "1") == "1":
                    zpA = psZ.tile([128, 2, 512], f32, tag="zpA")
                    zpB = psZ.tile([128, 2, 512], f32, tag="zpB")
                    zp4 = None
                else:
                    zp4 = psZ.tile([128, 4, 512], f32)

                def zsl(h):
                    if zp4 is not None:
                        return zp4[:, h, 0:WIN]
                    t = zpA if h < 2 else zpB
                    return t[:, h % 2, 0:WIN]

                def zslp(h):
                    if zp4 is not None:
                        return zp4[h * 32:(h + 1) * 32, h, 0:WIN]
                    t = zpA if h < 2 else zpB
                    return t[h * 32:(h + 1) * 32, h % 2, 0:WIN]
                if "mm" not in SKIP:
                    # Mask matmuls open the accumulation group so each slice
                    # is complete at its cc=1 matmul -- readers start sooner.
                    for h in range(n_act):
                        nc.tensor.matmul(
                            zsl(h),
                            id2_sb[:],
                            bm4_sb[:, h * WIN:(h + 1) * WIN],
                            start=True,
                            stop=False,
                            skip_group_check=True,
                        )
                    for cc in range(2):
                        for h in range(4):
                            nc.tensor.matmul(
                                zsl(h),
                                qmT_sb[:, cc, i * 128:(i + 1) * 128],
                                mp_t[:, cc, j * 1152 + h * WIN:j * 1152 + (h + 1) * WIN],
                                start=(cc == 0 and h >= n_act),
                                stop=(cc == 1),
                                skip_group_check=True,
                            )
                else:
                    for h in range(4):
                        nc.vector.memset(zsl(h), 0.0)
                z = zsb.tile([128, WIN], f32)
                if "stt" not in SKIP:
                    for h in range(n_act):
                        nc.scalar.copy(
                            out=z[h * 32:(h + 1) * 32, :],
                            in_=zslp(h),
                        )
                    for h in range(n_act, 4):
                        nc.vector.scalar_tensor_tensor(
                            out=z[h * 32:(h + 1) * 32, :],
                            in0=zslp(h),
                            scalar=1.0,
                            in1=bm_sb[h * 32:(h + 1) * 32, :],
                            op0=AL.mult, op1=AL.add,
                        )
                else:
                    nc.vector.memset(z[:], float(i))
                if "max" not in SKIP:
                    nc.vector.max(m8buf[:, i * 8:(i + 1) * 8], z[:])
                    nc.vector.max_index(
                        idx8[:, i * 8:(i + 1) * 8],
                        m8buf[:, i * 8:(i + 1) * 8], z[:],
                    )
                else:
                    nc.vector.memset(idx8[:, i * 8:(i + 1) * 8], 0)

        # Epilogue: decode argmax index -> (dy, dx) offsets.  Emitted in two
        # halves: the first half's decode + output DMAs run while the second
        # half of the stream is still computing, shortening the final chain.
        idxf = consts.tile([128, NCHUNK], f32)
        n_t = consts.tile([128, NCHUNK], f32)
        t3 = consts.tile([128, NCHUNK], f32)
        t6 = consts.tile([128, NCHUNK], f32)
        u_t = consts.tile([128, NCHUNK], f32)
        idx_top = idx8.rearrange("p (i e) -> p i e", e=8)[:, :, 0]
        out_r = out_d.rearrange("p (i t) -> p i t", t=2)

        def epilogue(lo, hi):
            sl = slice(lo, hi)
            nc.vector.tensor_copy(out=idxf[:, sl], in_=idx_top[:, sl])
            nc.vector.tensor_tensor(
                out=n_t[:, sl], in0=idxf[:, sl], in1=b9_sb[:, sl],
                op=AL.subtract,
            )                                                     # n in 0..8
            nc.vector.tensor_scalar(
                out=t3[:, sl], in0=n_t[:, sl], scalar1=3.0, scalar2=3.0,
                op0=AL.is_ge, op1=AL.mult,
            )                                                     # {0,3}
            nc.vector.tensor_scalar(
                out=t6[:, sl], in0=n_t[:, sl], scalar1=6.0, scalar2=3.0,
                op0=AL.is_ge, op1=AL.mult,
            )                                                     # {0,3}
            nc.vector.tensor_tensor(
                out=u_t[:, sl], in0=t3[:, sl], in1=t6[:, sl], op=AL.add
            )
            nc.vector.tensor_scalar(
                out=stage[:, sl, 0], in0=u_t[:, sl],
                scalar1=1.0 / 3.0, scalar2=-1.0,
                op0=AL.mult, op1=AL.add,
            )                                                     # dy = n//3 - 1
            nc.vector.scalar_tensor_tensor(
                out=stage[:, sl, 1], in0=n_t[:, sl], scalar=-1.0,
                in1=u_t[:, sl],
                op0=AL.add, op1=AL.subtract,
            )                                                     # dx = n%3 - 1
            dma_aux(out=out_r[:, sl, :], in_=stage[:, sl, :])
            # sync ring is free once its mp tiles are issued
            dma(out=m2_d[:, lo * 2:hi * 2],
                in_=m8buf.rearrange("p (i e) -> p i e", e=8)[:, lo:hi, 0:2])

        epilogue(0, NCHUNK // 2)
        epilogue(NCHUNK // 2, NCHUNK)


def _get_nc():
    if "nc" not in _CACHE:
        _CACHE["nc"] = _build_nc()
    return _CACHE["nc"]


def make_in_maps(HSI_Patch, MSI_Patch2, W_qkv1, W_qkv2):
    hp = np.asarray(HSI_Patch, np.float32).reshape(B, C, L)
    mp = np.asarray(MSI_Patch2, np.float32).reshape(B, C, 9 * L)
    # Host pre-transposes the small weight slices: (k, c) layout feeds the
    # tensor-engine contraction directly, skipping on-device transposes.
    wqt = np.ascontiguousarray(np.asarray(W_qkv1, np.float32)[:, :C].T)
    wkt = np.ascontiguousarray(np.asarray(W_qkv2, np.float32)[:, C:2 * C].T)

    bm = np.full((128, WIN), NEG, np.float32)
    for d in range(32):
        bm[d::32, 9 * d:9 * d + 9] = 0.0
    # Per-slice mask: slice h only keeps rows p//32 == h, all else killed.
    # Only slices 0..1 take the matmul-mask path (KACTH=2).
    bm4 = np.full((128, 2 * WIN), NEG, np.float32)
    for h in range(2):
        for d in range(32):
            bm4[32 * h + d, h * WIN + 9 * d:h * WIN + 9 * d + 9] = 0.0
    ident = np.eye(128, dtype=np.float32)
    b9 = np.ascontiguousarray(
        np.broadcast_to(
            (9.0 * (np.arange(128) % 32)).astype(np.float32)[:, None], (128, NCHUNK)
        )
    )

    in_maps = []
    for core in range(8):
        b, half = core // 2, core % 2
        in_maps.append({
            "hp": np.ascontiguousarray(hp[b, :, half * L_SH:(half + 1) * L_SH]),
            "mp": np.ascontiguousarray(mp[b, :, half * S_SH:(half + 1) * S_SH]),
            "wqt": wqt,
            "wkt": wkt,
            "bandmask": bm,
            "bandmask4": bm4,
            "ident_r": ident,
            "base9": b9,
        })
    return in_maps


def gather_out(results):
    out = np.zeros((B, L, 2), np.float32)
    gap = np.zeros((B, L), np.float32)
    for core in range(8):
        b, half = core // 2, core % 2
        r = np.asarray(results[core]["out"], np.float32)  # (128, 32)
        r = r.reshape(128, NCHUNK, 2).transpose(1, 0, 2).reshape(L_SH, 2)
        out[b, half * L_SH:(half + 1) * L_SH] = r
        g = np.asarray(results[core]["m2"], np.float32)
        g = g.reshape(128, NCHUNK, 2).transpose(1, 0, 2).reshape(L_SH, 2)
        gap[b, half * L_SH:(half + 1) * L_SH] = g[:, 0] - g[:, 1]
    return out, gap


# Pixels whose top-2 attention gap is below this get an exact float64
# re-resolve on the host (fp32r matmul noise is ~1e-4; 40x safety margin).
GAP_TAU = 4e-3


def refine_ties(out, gap, HSI_Patch, MSI_Patch2, W_qkv1, W_qkv2):
    risky = np.argwhere(gap < GAP_TAU)
    if risky.size == 0:
        return out
    hp = np.asarray(HSI_Patch, np.float64).reshape(B, C, L)
    mp = np.asarray(MSI_Patch2, np.float64).reshape(B, C, 9 * L)
    Wq = np.asarray(W_qkv1, np.float64)[:, :C]
    Wk = np.asarray(W_qkv2, np.float64)[:, C:2 * C]
    for b, l in risky:
        q = hp[b, :, l] @ Wq
        k9 = mp[b, :, 9 * l:9 * l + 9].T @ Wk        # (9, C)
        n = int(np.argmax(k9 @ q))
        out[b, l, 0] = n // 3 - 1
        out[b, l, 1] = n % 3 - 1
    return out


def kernel(x, y, HSI_Patch, MSI_Patch2, W_qkv1, W_qkv2, **_unused):
    import time

    from concourse.bass_utils import run_bass_kernel_spmd

    nc = _get_nc()
    in_maps = make_in_maps(HSI_Patch, MSI_Patch2, W_qkv1, W_qkv2)
    # A freshly-acquired NeuronCore occasionally reports a transient
    # NRT_EXEC_UNIT_UNRECOVERABLE from a previous tenant's aborted run;
    # a retry after a short pause recovers it.
    last_exc = None
    for attempt in range(3):
        try:
            res = run_bass_kernel_spmd(nc, in_maps, core_ids=list(range(8)))
            break
        except Exception as e:  # noqa: BLE001 -- retry only transient NRT states
            last_exc = e
            if "UNRECOVERABLE" not in str(e) and "UNAVAILABLE" not in str(e):
                raise
            time.sleep(5 * (attempt + 1))
    else:
        raise last_exc
    out, gap = gather_out(res.results)
    out = refine_ties(out, gap, HSI_Patch, MSI_Patch2, W_qkv1, W_qkv2)
    return out.reshape(B, H, H, 2)



# revision 3
# speedup vs baseline: 2.2944x; 2.2944x over previous
"""Trainium2 Bass kernel for nn_Attention_FRN (sparse windowed attention argmax).

Math: reference computes
    q  = (HSI flat -> (B,L,C)) @ Wq          (Wq = W_qkv1[:, :C])
    k  = (MSI flat -> (B,9L,C)) @ Wk         (Wk = W_qkv2[:, C:2C])
    attn[b,l,n] = <q[b,l], k[b,9l+n]> * scale
    out = argmax_n softmax(attn)  -> (n//3-1, n%3-1) offsets, reshaped (B,H,H,2)

Softmax and the positive scale are argmax-invariant, so both are skipped.
x and y inputs are unused (only x's shape matters).  The big k projection is
eliminated algebraically:  <q[l], mp[s] @ Wk> = <(q @ Wk^T)[l], mp[s]>, so we
only project the small hp tensor:  qm = (hp @ Wq) @ Wk^T, then
attn[l, n] = <qm[l], mp[9l+n]>.

Device mapping (cost-model-shaped):
  * mp is streamed in fp8e4 (4.7MB/core), the DMA roofline term; hp/weights
    in fp16.  The PE matmul allows mixed dtypes, so the moving operand stays
    fp16 (qm) against an fp8 stationary (mp) -- fp8 noise only from mp.
  * Banded attn is computed windows-on-partitions: stationary = mp window
    (126 cols = 14 pixels x 9 offsets), moving = 14 qm pixel columns.  Each
    pack of 126 pixels = 9 groups accumulated into one PSUM tile [128,126]
    on top of an identity-matmul band mask (NEG off-band), then ACT-copied
    to SBUF fp16, PE-transposed to [126,128] (pixel partitions), and DVE
    max/max_index give per-pixel top-8 values + argmax position.
  * Top-2 gap ships to the host; pixels with gap < GAP_TAU (fp8 noise
    ~5 sigma) are exactly re-resolved on host in float64.

Sharding: 8 cores = B(4) x L-half(2).  Per core: hp shard (256, 2048),
mp shard (256, 18432), replicated weights + small constants.
"""

import numpy as np

B, C, H = 4, 256, 64
L = H * H                  # 4096 pixels per batch
L_SH = L // 2              # 2048 per core
S_SH = 9 * L_SH            # 18432 mp columns per core
PACK = 126                 # pixels per pack (9 groups of 14)
NPACKS = 17                # 16 full packs + one 32-pixel tail
N_TILES = 16               # mp tiles: 15 x 1134 cols + final 1422
TILE_W = 1134              # 126 pixels' windows
NEG = -60000.0             # off-band mask value; fp16-safe

_CACHE = {}


def _build_nc():
    import concourse.bacc as bacc
    import concourse.tile as tile
    from concourse import mybir

    f32 = mybir.dt.float32
    f16 = mybir.dt.float16
    f8 = mybir.dt.float8e4
    u16 = mybir.dt.uint16

    nc = bacc.Bacc(
        "TRN2",
        target_bir_lowering=False,
        debug=False,
        enable_asserts=False,
        num_devices=8,
    )
    mp_d = nc.dram_tensor("mp", [128, 2, S_SH], f8, kind="ExternalInput").ap()
    hp_d = nc.dram_tensor("hp", [128, 2, L_SH], f16, kind="ExternalInput").ap()
    wqt_d = nc.dram_tensor("wqt", [128, 2, C], f16, kind="ExternalInput").ap()
    wkt_d = nc.dram_tensor("wkt", [128, 2, C], f16, kind="ExternalInput").ap()
    id_d = nc.dram_tensor("ident", [128, 128], f16, kind="ExternalInput").ap()
    mk_d = nc.dram_tensor("maskT", [128, PACK], f16, kind="ExternalInput").ap()
    nb_d = nc.dram_tensor("nbase", [128, NPACKS], f32, kind="ExternalInput").ap()
    out_d = nc.dram_tensor("outo", [128, NPACKS, 2], f32, kind="ExternalOutput").ap()
    m2_d = nc.dram_tensor("m2o", [128, NPACKS, 2], f32, kind="ExternalOutput").ap()

    with tile.TileContext(nc) as tc:
        _emit(tc, out_d, m2_d, mp_d, hp_d, wqt_d, wkt_d, id_d, mk_d, nb_d)
    nc.compile()
    return nc


def _emit(tc, out_d, m2_d, mp_d, hp_d, wqt_d, wkt_d, id_d, mk_d, nb_d):
    from contextlib import ExitStack

    from concourse import mybir

    nc = tc.nc
    f32 = mybir.dt.float32
    f16 = mybir.dt.float16
    f8 = mybir.dt.float8e4
    u16 = mybir.dt.uint16
    AL = mybir.AluOpType
    dma = nc.sync.dma_start          # SP HWDGE ring: the mp stream
    dma_aux = nc.scalar.dma_start    # ACT HWDGE ring: consts/hp/out

    with ExitStack() as ctx:
        consts = ctx.enter_context(tc.tile_pool(name="consts", bufs=1))
        mpp = ctx.enter_context(tc.tile_pool(name="mpp", bufs=N_TILES))
        ztp = ctx.enter_context(tc.tile_pool(name="ztp", bufs=3))
        setup_ctx = ExitStack()
        psM = setup_ctx.enter_context(tc.tile_pool(name="psM", bufs=2, space="PSUM"))
        psQ = setup_ctx.enter_context(tc.tile_pool(name="psQ", bufs=2, space="PSUM"))

        wqt_sb = consts.tile([128, 2, C], f16)    # (k%128, k//128, c1)
        wkt_sb = consts.tile([128, 2, C], f16)    # (k%128, k//128, c2)
        id_sb = consts.tile([128, 128], f16)
        mk_sb = consts.tile([128, PACK], f16)
        nb_sb = consts.tile([128, NPACKS], f32)
        hp_sb = consts.tile([128, 2, L_SH], f16)  # (c1%128, c1//128, l)
        m_sb = consts.tile([128, 2, C], f16)      # (c1%128, c1//128, c2)
        qmT_sb = consts.tile([128, 2, L_SH], f16)  # (c2%128, c2//128, l)
        m8buf = consts.tile([128, NPACKS, 8], f32)
        idxb = consts.tile([128, NPACKS, 8], u16)
        stage = consts.tile([128, NPACKS, 2], f32)
        m2s = consts.tile([128, NPACKS, 2], f32)

        # Weight/identity DMAs unblock the M chain; hp unblocks qmT; the
        # mask/base constants aren't needed until pack 0 / the epilogue.
        dma_aux(out=wqt_sb[:], in_=wqt_d)
        dma_aux(out=wkt_sb[:], in_=wkt_d)
        dma_aux(out=hp_sb[:], in_=hp_d)
        dma_aux(out=id_sb[:], in_=id_d)
        dma_aux(out=mk_sb[:], in_=mk_d)
        dma_aux(out=nb_sb[:], in_=nb_d)

        # mp stream: 15 tiles of 1134 cols + final 1422 (pack 15 and the
        # 32-pixel tail pack share tile 15, so windows never span tiles).
        mp_t = []
        for t in range(N_TILES):
            w = TILE_W if t < N_TILES - 1 else S_SH - TILE_W * (N_TILES - 1)
            mt = mpp.tile([128, 2, w], f8, name=f"mp{t}", tag="mp_t")
            dma(out=mt[:], in_=mp_d[:, :, TILE_W * t:TILE_W * t + w])
            mp_t.append(mt)

        # M[c1, c2] = sum_k Wq[c1, k] * Wk[c2, k]  (so attn = <hp @ M, mp>)
        for c1h in range(2):
            mps = psM.tile([128, C], f32, tag="mps")
            for kk in range(2):
                nc.tensor.matmul(
                    mps[:],
                    wqt_sb[:, kk, c1h * 128:(c1h + 1) * 128],
                    wkt_sb[:, kk, :],
                    start=(kk == 0),
                    stop=(kk == 1),
                )
            nc.scalar.copy(out=m_sb[:, c1h, :], in_=mps[:])

        # qmT[c2, l] = sum_c1 M[c1, c2] * hpT[c1, l], emitted l-block-major
        # so pack 0's qm columns are ready after the first two blocks.
        for ls in range(4):
            for cc2 in range(2):
                qp = psQ.tile([128, 512], f32, tag="qp")
                for c1h in range(2):
                    nc.tensor.matmul(
                        qp[:],
                        m_sb[:, c1h, cc2 * 128:(cc2 + 1) * 128],
                        hp_sb[:, c1h, ls * 512:(ls + 1) * 512],
                        start=(c1h == 0),
                        stop=(c1h == 1),
                    )
                nc.scalar.copy(out=qmT_sb[:, cc2, ls * 512:(ls + 1) * 512], in_=qp[:])

        setup_ctx.close()
        psA = ctx.enter_context(tc.tile_pool(name="psA", bufs=3, space="PSUM"))
        psB = ctx.enter_context(tc.tile_pool(name="psB", bufs=3, space="PSUM"))

        def epilogue(lo, hi):
            """Decode argmax position -> (dy, dx) for packs [lo, hi)."""
            sl = slice(lo, hi)
            p_ = slice(0, PACK)
            nc.vector.tensor_copy(out=idxf[p_, sl], in_=idxb[p_, sl, 0])
            nc.vector.tensor_tensor(
                out=n_t[p_, sl], in0=idxf[p_, sl], in1=nb_sb[p_, sl],
                op=AL.subtract,
            )                                                     # n in 0..8
            nc.vector.tensor_scalar(
                out=t3[p_, sl], in0=n_t[p_, sl], scalar1=3.0, scalar2=3.0,
                op0=AL.is_ge, op1=AL.mult,
            )                                                     # {0,3}
            nc.vector.tensor_scalar(
                out=t6[p_, sl], in0=n_t[p_, sl], scalar1=6.0, scalar2=3.0,
                op0=AL.is_ge, op1=AL.mult,
            )                                                     # {0,3}
            nc.vector.tensor_tensor(
                out=u_t[p_, sl], in0=t3[p_, sl], in1=t6[p_, sl], op=AL.add
            )
            nc.vector.tensor_scalar(
                out=stage[p_, sl, 0], in0=u_t[p_, sl],
                scalar1=1.0 / 3.0, scalar2=-1.0,
                op0=AL.mult, op1=AL.add,
            )                                                     # dy = n//3 - 1
            nc.vector.scalar_tensor_tensor(
                out=stage[p_, sl, 1], in0=n_t[p_, sl], scalar=-1.0,
                in1=u_t[p_, sl],
                op0=AL.add, op1=AL.subtract,
            )                                                     # dx = n%3 - 1
            nc.vector.tensor_copy(out=m2s[p_, sl, :], in_=m8buf[p_, sl, 0:2])
            dma_aux(out=out_d[0:PACK, sl, :], in_=stage[p_, sl, :])
            dma(out=m2_d[0:PACK, sl, :], in_=m2s[p_, sl, :])

        idxf = consts.tile([128, NPACKS], f32)
        n_t = consts.tile([128, NPACKS], f32)
        t3 = consts.tile([128, NPACKS], f32)
        t6 = consts.tile([128, NPACKS], f32)
        u_t = consts.tile([128, NPACKS], f32)

        # Main loop: one pack of 126 pixels (32 for the tail) per iteration.
        for k in range(NPACKS):
            npx = PACK if k < NPACKS - 1 else L_SH - PACK * (NPACKS - 1)
            t_idx = min(k, N_TILES - 1)
            loc = TILE_W * (k - t_idx)   # 0, or 1134 for the tail pack
            za = psA.tile([128, PACK], f32, tag="za")
            # Band mask lands first (opens the accumulation group): pixel
            # column m gets 0 at rows 9*(m%14)..+9, NEG elsewhere.
            nc.tensor.matmul(
                za[:],
                id_sb[:],
                mk_sb[:],
                start=True,
                stop=False,
                skip_group_check=True,
            )
            j0 = 0
            while j0 < npx:
                pg = min(14, npx - j0)
                for cc in range(2):
                    last = (j0 + pg >= npx) and cc == 1
                    nc.tensor.matmul(
                        za[0:9 * pg, j0:j0 + pg],
                        mp_t[t_idx][:, cc, loc + 9 * j0:loc + 9 * (j0 + pg)],
                        qmT_sb[:, cc, PACK * k + j0:PACK * k + j0 + pg],
                        start=False,
                        stop=last,
                        skip_group_check=True,
                    )
                j0 += pg
            zt = ztp.tile([128, PACK], f16, tag="zt")
            nc.scalar.copy(out=zt[:], in_=za[:])
            zb = psB.tile([PACK, 128], f16, tag="zb")
            nc.tensor.transpose(zb[:], zt[:], id_sb[:])
            nc.vector.max(m8buf[0:PACK, k, :], zb[:])
            nc.vector.max_index(idxb[0:PACK, k, :], m8buf[0:PACK, k, :], zb[:])
            if k == 8:
                epilogue(0, 9)
        epilogue(9, NPACKS)


def _get_nc():
    if "nc" not in _CACHE:
        _CACHE["nc"] = _build_nc()
    return _CACHE["nc"]


def make_in_maps(HSI_Patch, MSI_Patch2, W_qkv1, W_qkv2):
    import ml_dtypes

    f8 = ml_dtypes.float8_e4m3fn
    hp = np.asarray(HSI_Patch, np.float32).reshape(B, C, L)
    mp = np.asarray(MSI_Patch2, np.float32).reshape(B, C, 9 * L)
    # (c, ...) -> (c%128, c//128, ...) partition layout, host-side cast.
    wqt = np.ascontiguousarray(
        np.asarray(W_qkv1, np.float32)[:, :C].T.reshape(2, 128, C)
        .transpose(1, 0, 2)).astype(np.float16)
    wkt = np.ascontiguousarray(
        np.asarray(W_qkv2, np.float32)[:, C:2 * C].T.reshape(2, 128, C)
        .transpose(1, 0, 2)).astype(np.float16)

    ident = np.eye(128, dtype=np.float16)
    maskT = np.full((128, PACK), NEG, np.float16)
    for m in range(PACK):
        j = m % 14
        maskT[9 * j:9 * j + 9, m] = 0.0
    nbase = np.broadcast_to(
        (9.0 * (np.arange(128) % 14)).astype(np.float32)[:, None], (128, NPACKS)
    )
    nbase = np.ascontiguousarray(nbase)

    in_maps = []
    for core in range(8):
        b, half = core // 2, core % 2
        hp_sh = hp[b, :, half * L_SH:(half + 1) * L_SH]
        mp_sh = mp[b, :, half * S_SH:(half + 1) * S_SH]
        in_maps.append({
            "mp": np.ascontiguousarray(
                mp_sh.reshape(2, 128, S_SH).transpose(1, 0, 2)).astype(f8),
            "hp": np.ascontiguousarray(
                hp_sh.reshape(2, 128, L_SH).transpose(1, 0, 2)).astype(np.float16),
            "wqt": wqt,
            "wkt": wkt,
            "ident": ident,
            "maskT": maskT,
            "nbase": nbase,
        })
    return in_maps


def gather_out(results):
    out = np.zeros((B, L, 2), np.float32)
    gap = np.zeros((B, L), np.float32)
    for core in range(8):
        b, half = core // 2, core % 2
        r = np.asarray(results[core]["outo"], np.float32)   # (128, 17, 2)
        g = np.asarray(results[core]["m2o"], np.float32)    # (128, 17, 2)
        for k in range(NPACKS):
            npx = PACK if k < NPACKS - 1 else L_SH - PACK * (NPACKS - 1)
            lo = half * L_SH + PACK * k
            out[b, lo:lo + npx] = r[0:npx, k]
            gap[b, lo:lo + npx] = g[0:npx, k, 0] - g[0:npx, k, 1]
    return out, gap


# Pixels whose top-2 attention gap is below this get an exact float64
# re-resolve on the host.  fp8(mp) noise on an attn value is ~0.06; the
# top-2 gap noise is ~0.083, so 0.35 is a ~4.2 sigma guard band.
GAP_TAU = 0.35


def refine_ties(out, gap, HSI_Patch, MSI_Patch2, W_qkv1, W_qkv2):
    risky = np.argwhere(gap < GAP_TAU)
    if risky.size == 0:
        return out
    hp = np.asarray(HSI_Patch, np.float64).reshape(B, C, L)
    mp = np.asarray(MSI_Patch2, np.float64).reshape(B, C, 9 * L)
    Wq = np.asarray(W_qkv1, np.float64)[:, :C]
    Wk = np.asarray(W_qkv2, np.float64)[:, C:2 * C]
    M = Wq @ Wk.T
    rb, rl = risky[:, 0], risky[:, 1]
    qm = np.einsum("rc,cd->rd", hp[rb, :, rl], M)            # (R, C)
    win = (9 * rl)[:, None] + np.arange(9)[None, :]          # (R, 9)
    k9 = mp[rb[:, None], :, win]                             # (R, 9, C)
    n = np.einsum("rnc,rc->rn", k9, qm).argmax(1)
    out[rb, rl, 0] = n // 3 - 1
    out[rb, rl, 1] = n % 3 - 1
    return out


def kernel(x, y, HSI_Patch, MSI_Patch2, W_qkv1, W_qkv2, **_unused):
    import time

    from concourse.bass_utils import run_bass_kernel_spmd

    nc = _get_nc()
    in_maps = make_in_maps(HSI_Patch, MSI_Patch2, W_qkv1, W_qkv2)
    # A freshly-acquired NeuronCore occasionally reports a transient
    # NRT_EXEC_UNIT_UNRECOVERABLE from a previous tenant's aborted run;
    # a retry after a short pause recovers it.
    last_exc = None
    for attempt in range(3):
        try:
            res = run_bass_kernel_spmd(nc, in_maps, core_ids=list(range(8)))
            break
        except Exception as e:  # noqa: BLE001 -- retry only transient NRT states
            last_exc = e
            if "UNRECOVERABLE" not in str(e) and "UNAVAILABLE" not in str(e):
                raise
            time.sleep(5 * (attempt + 1))
    else:
        raise last_exc
    out, gap = gather_out(res.results)
    out = refine_ties(out, gap, HSI_Patch, MSI_Patch2, W_qkv1, W_qkv2)
    return out.reshape(B, H, H, 2)
